# revision 1
# baseline (speedup 1.0000x reference)
"""Multi-head causal attention (B=2, L=2048, H=2048, NH=16) on 8 Trainium2
NeuronCores.

Sharding: tensor-parallel over heads — core c computes heads {2c, 2c+1}.
Each core:
  phase 1: q/k/v projections for its 256 output dims (contract over H=2048)
  phase 2: causal attention for its 2 heads + its partial o-projection
Host: transposes/rounds inputs (fp32r layout prep), sums the 8 partial
o-projection outputs, and transposes back.

All matmuls run in float32r (fp32 with 11-bit mantissa, 1 cycle/row on the
PE for free dims >= 256 — 4x faster than plain fp32 at ~2.4e-4 rounding).

Phase-2 softmax is structured to keep the PE dense (HAM stays warm):
  - colsum of exp accumulates on the PE via a ones-matmul per j-tile
    (PSUM accumulation), not a DVE add chain
  - reciprocal runs on a single (1 x 512) row, then gpsimd
    partition_broadcast replicates it
  - o-projection of chunk N is emitted after attention of chunk N+1 so the
    PE never waits for the softmax normalize chain
  - causally-masked j-tiles are skipped; diagonal j-tiles stream only the
    live i-columns (floor 256 — below that fp32r drops to 4 cyc/row)
"""

import os
import sys

if "/opt/trn_rl_repo" not in sys.path:
    sys.path.insert(0, "/opt/trn_rl_repo")

import numpy as np

from concourse import bacc, mybir, tile  # noqa: E402
from concourse.bass_utils import run_bass_kernel_spmd  # noqa: E402

F32R = mybir.dt.float32r
F32 = mybir.dt.float32

N_CORES = 8
B, L, H, NH = 2, 2048, 2048, 16
DH = H // NH                       # 128
BL = B * L                        # 4096
HPC = NH // N_CORES               # heads per core = 2
OPC = HPC * DH                    # output dims per core = 256
HT = H // 128                     # 16 h-tiles (contraction)
IC1 = 256                         # phase-1 i-chunk width
N_IC1 = BL // IC1                 # 16
IC2 = 512                         # phase-2 i-chunk width
N_IC2 = L // IC2                  # 4 per batch
JT = L // 128                     # 16 j-tiles per batch
SCALE = 1.0 / float(np.sqrt(DH))

LAST_EXEC_NS = None


def _round_fp32r(a: np.ndarray) -> np.ndarray:
    """Round fp32 to fp32r (11-bit mantissa, round-to-nearest-even)."""
    a = np.ascontiguousarray(a, dtype=np.float32)
    u = a.view(np.uint32)
    low = u & np.uint32(0xFFF)
    rounded = (u & np.uint32(0xFFFFF000)).astype(np.uint64)
    half = np.uint32(0x800)
    lsb = (u >> np.uint32(12)) & np.uint32(1)
    up = (low > half) | ((low == half) & (lsb == 1))
    rounded = rounded + (up.astype(np.uint64) << np.uint64(12))
    return rounded.astype(np.uint32).view(np.float32).reshape(a.shape)


def _build():
    nc = bacc.Bacc(None, target_bir_lowering=False, debug=False)

    xt = nc.declare_dram_parameter("xt", [H, BL], F32R, isOutput=False)
    wq = nc.declare_dram_parameter("wq", [H, OPC], F32R, isOutput=False)
    wk = nc.declare_dram_parameter("wk", [H, OPC], F32R, isOutput=False)
    wv = nc.declare_dram_parameter("wv", [H, OPC], F32R, isOutput=False)
    wo = nc.declare_dram_parameter("wo", [OPC, H], F32R, isOutput=False)
    out = nc.declare_dram_parameter("out", [H, BL], F32, isOutput=True)

    with tile.TileContext(nc) as tc:
        with tc.tile_pool(name="persist", bufs=1) as persist, \
             tc.tile_pool(name="psum", bufs=8, space="PSUM") as psum:
            qt_sb = persist.tile([128, HPC, BL], F32R, tag="qt")
            kt_sb = persist.tile([128, HPC, BL], F32R, tag="kt")
            v_sb = persist.tile([128, BL // 128, OPC], F32R, tag="v")
            ones_sb = persist.tile([128, 128], F32R, tag="ones")

            # ---------------- phase 1: q/k/v projections ----------------
            with tc.tile_pool(name="wpool", bufs=1) as wpool, \
                 tc.tile_pool(name="xpool", bufs=2) as xpool, \
                 tc.tile_pool(name="misc1", bufs=1) as misc1:
                wq_sb = wpool.tile([128, HT, OPC], F32R, tag="wq")
                wk_sb = wpool.tile([128, HT, OPC], F32R, tag="wk")
                wv_sb = wpool.tile([128, HT, OPC], F32R, tag="wv")
                # Fine-grained startup DMAs alternating across the two
                # HWDGE queues so the first matmul's inputs land fast.
                xchs = {}
                xchs[0] = xpool.tile([128, HT, IC1], F32R, tag="xch",
                                     name="xch")
                def dma_split(dst3, src2d):
                    # halve a [128, T, F]-tile transfer across both HW
                    # queues — one queue sustains only ~170 GB/s
                    t = dst3.shape[1]
                    r = src2d.rearrange("(q t p) f -> q p t f", q=2, p=128)
                    nc.sync.dma_start(out=dst3[:, :t // 2, :], in_=r[0])
                    nc.scalar.dma_start(out=dst3[:, t // 2:, :], in_=r[1])

                # wq rides the gpsimd SWDGE queue (third stream) so the
                # two HWDGE queues spend the ramp on x/wk/wv; per-ht
                # pieces keep q-proj supplied as they land
                for ht in range(HT):
                    nc.gpsimd.dma_start(
                        out=wq_sb[:, ht, :],
                        in_=wq[ht * 128:(ht + 1) * 128, :])
                dma_split(xchs[0], xt[:, 0:IC1])
                dma_split(wk_sb, wk[:, :])
                dma_split(wv_sb, wv[:, :])

                ones_f = misc1.tile([128, 128], F32)
                nc.vector.memset(ones_f[:, :], 1.0)
                nc.vector.tensor_copy(ones_sb[:, :], ones_f[:, :])

                for ic in range(N_IC1):
                    if ic not in xchs:
                        xchs[ic] = xpool.tile([128, HT, IC1], F32R,
                                              tag="xch", name="xch")
                        dma_split(xchs[ic],
                                  xt[:, ic * IC1:(ic + 1) * IC1])
                    xch = xchs.pop(ic)
                    # q^T and k^T: (o_local x i), stationary = W^T h-tiles
                    ncopy = 0
                    for wsb, dest in ((wq_sb, qt_sb), (wk_sb, kt_sb)):
                        for ot in range(HPC):
                            ps = psum.tile([128, IC1], F32, tag="bank",
                                           name="ps",
                                           padded_shape=[128, IC2])
                            for ht in range(HT):
                                nc.tensor.matmul(
                                    ps[:, :],
                                    wsb[:, ht, ot * 128:(ot + 1) * 128],
                                    xch[:, ht, :],
                                    start=(ht == 0), stop=(ht == HT - 1))
                            if ncopy % 2 == 0:
                                nc.scalar.copy(
                                    dest[:, ot, ic * IC1:(ic + 1) * IC1],
                                    ps[:, :])
                            else:
                                nc.vector.tensor_copy(
                                    dest[:, ot, ic * IC1:(ic + 1) * IC1],
                                    ps[:, :])
                            ncopy += 1
                    # v in natural (j x o) layout, stationary = x^T tiles
                    for it in range(IC1 // 128):
                        ps = psum.tile([128, OPC], F32, tag="bank",
                                       name="ps", padded_shape=[128, IC2])
                        for ht in range(HT):
                            nc.tensor.matmul(
                                ps[:, :],
                                xch[:, ht, it * 128:(it + 1) * 128],
                                wv_sb[:, ht, :],
                                start=(ht == 0), stop=(ht == HT - 1))
                        if it % 2 == 0:
                            nc.scalar.copy(
                                v_sb[:, ic * (IC1 // 128) + it, :], ps[:, :])
                        else:
                            nc.vector.tensor_copy(
                                v_sb[:, ic * (IC1 // 128) + it, :], ps[:, :])

            # ---------- phase 2: attention + pipelined o-projection ----------
            with tc.tile_pool(name="wo_pool", bufs=1) as wo_pool, \
                 tc.tile_pool(name="exp_pool", bufs=4) as exp_pool, \
                 tc.tile_pool(name="sm_pool", bufs=4) as sm_pool, \
                 tc.tile_pool(name="mst_pool", bufs=3) as mst_pool, \
                 tc.tile_pool(name="oc_pool", bufs=6) as oc_pool:
                wo_sb = wo_pool.tile([128, HPC, H], F32R, tag="wo")
                nc.scalar.dma_start(
                    out=wo_sb[:, :, :],
                    in_=wo[:, :].rearrange("(t p) f -> p t f", p=128))

                def emit_oproj_ot(mst, gio, ot):
                    op = psum.tile([128, IC2], F32, tag="bank", name="op")
                    for hh in range(HPC):
                        nc.tensor.matmul(
                            op[:, :],
                            wo_sb[:, hh, ot * 128:(ot + 1) * 128],
                            mst[:, hh, :],
                            start=(hh == 0), stop=(hh == HPC - 1))
                    oc = oc_pool.tile([128, IC2], F32, tag="oc", name="oc")
                    nc.vector.tensor_copy(oc[:, :], op[:, :])
                    eng = nc.sync if ot % 2 == 0 else nc.scalar
                    eng.dma_start(
                        out=out[ot * 128:(ot + 1) * 128, gio:gio + IC2],
                        in_=oc[:, :])

                pending = None
                for b in range(B):
                    for ic in range(N_IC2):
                        gio = b * L + ic * IC2
                        njt = 4 * ic + 4      # causal: j-tiles 0..4ic+3
                        mst = mst_pool.tile([128, HPC, IC2], F32R, tag="mst",
                                            name="mst")
                        # spread the previous chunk's 16 o-proj groups
                        # across this chunk's 2*njt j-tile iterations so the
                        # PE has fill work while waiting on ACT exp results
                        filler = []
                        if pending is not None:
                            pmst, pgio = pending
                            filler = [(pmst, pgio, ot)
                                      for ot in range(H // 128)]
                        fill_idx = 0
                        total_jts = HPC * njt
                        jt_counter = 0
                        for h in range(HPC):
                            mx = psum.tile([128, IC2], F32, tag="bank",
                                           name="mx")
                            rs = psum.tile([1, IC2], F32, tag="bank",
                                           name="rs", padded_shape=[128, IC2])
                            for jt in range(njt):
                                jt_counter += 1
                                # live i-columns: i >= j on diagonal tiles;
                                # keep width >= 256 for fp32r full rate
                                f0 = min(max(0, 128 * jt - IC2 * ic), IC2 - 256)
                                w = IC2 - f0
                                sc = psum.tile([128, IC2], F32, tag="bank",
                                               name="sc")
                                nc.tensor.matmul(
                                    sc[:, f0:],
                                    kt_sb[:, h, b * L + jt * 128:
                                          b * L + (jt + 1) * 128],
                                    qt_sb[:, h, gio + f0:gio + IC2],
                                    start=True, stop=True)
                                ex = exp_pool.tile([128, IC2], F32R, tag="ex")
                                nc.scalar.activation(
                                    ex[:, f0:], sc[:, f0:],
                                    mybir.ActivationFunctionType.Exp,
                                    scale=SCALE)
                                if jt >= 4 * ic:
                                    # zero where j > i
                                    nc.gpsimd.affine_select(
                                        ex[:, f0:], ex[:, f0:],
                                        pattern=[[1, w]],
                                        compare_op=mybir.AluOpType.is_ge,
                                        fill=0.0,
                                        base=f0 - (128 * jt - IC2 * ic),
                                        channel_multiplier=-1)
                                nc.tensor.matmul(
                                    rs[:, f0:], ones_sb[:, 0:1], ex[:, f0:],
                                    start=(jt == 0), stop=(jt == njt - 1))
                                nc.tensor.matmul(
                                    mx[:, f0:],
                                    v_sb[:, b * JT + jt,
                                         h * 128:(h + 1) * 128],
                                    ex[:, f0:],
                                    start=(jt == 0), stop=(jt == njt - 1))
                                want = (jt_counter * len(filler)) // total_jts
                                while fill_idx < want:
                                    emit_oproj_ot(*filler[fill_idx])
                                    fill_idx += 1
                            rec_row = sm_pool.tile([1, IC2], F32, tag="recrow")
                            nc.vector.reciprocal_approx_fast(
                                out=rec_row[:, :], in_=rs[0:1, :])
                            rec_sb = sm_pool.tile([128, IC2], F32, tag="recb")
                            nc.gpsimd.partition_broadcast(
                                rec_sb[:, :], rec_row[:, :], channels=128)
                            nc.vector.tensor_mul(mst[:, h, :], mx[:, :],
                                                 rec_sb[:, :])
                        while fill_idx < len(filler):
                            emit_oproj_ot(*filler[fill_idx])
                            fill_idx += 1
                        pending = (mst, gio)
                pmst, pgio = pending
                for ot in range(H // 128):
                    emit_oproj_ot(pmst, pgio, ot)
    nc.finalize()
    return nc


_NC_CACHE = None


def _get_nc():
    global _NC_CACHE
    if _NC_CACHE is None:
        _NC_CACHE = _build()
    return _NC_CACHE


def _install_hook_shim():
    """Make antenv.axon_hooks importable (absent on this image) so
    run_bass_kernel_spmd's trace path degrades gracefully."""
    import types
    import antenv
    if "antenv.axon_hooks" not in sys.modules:
        shim = types.ModuleType("antenv.axon_hooks")

        def set_axon_ntff_profile_hook(h):
            shim._the_hook = h

        def get_axon_ntff_profile_hook():
            return getattr(shim, "_the_hook", None)

        shim.set_axon_ntff_profile_hook = set_axon_ntff_profile_hook
        shim.get_axon_ntff_profile_hook = get_axon_ntff_profile_hook
        sys.modules["antenv.axon_hooks"] = shim
        antenv.axon_hooks = shim


def _enable_profiling():
    """Wire the axon NTFF profile hook for neuron-profile timing."""
    _install_hook_shim()
    from trn_agent_boot.trn_boot import _ntff_profile_via_ctypes
    hook = _ntff_profile_via_ctypes("/opt/axon/libaxon_pjrt.so")
    sys.modules["antenv.axon_hooks"].set_axon_ntff_profile_hook(hook)
    import concourse.bass_utils as bu
    bu.upload_artifacts = lambda tmpdir: "local://" + tmpdir


def kernel(x, padding_mask, Wq, Wk, Wv, Wo):
    global LAST_EXEC_NS
    x = np.asarray(x, dtype=np.float32)
    Wq = np.asarray(Wq, dtype=np.float32)
    Wk = np.asarray(Wk, dtype=np.float32)
    Wv = np.asarray(Wv, dtype=np.float32)
    Wo = np.asarray(Wo, dtype=np.float32)

    xt = _round_fp32r(x.reshape(BL, H).T)        # (H, BL)
    wqt = _round_fp32r(Wq.T)                     # (H, H): [h, o]
    wkt = _round_fp32r(Wk.T)
    wvt = _round_fp32r(Wv.T)
    wot = _round_fp32r(Wo.T)                     # (H, H): [h_in, o]

    in_maps = []
    for c in range(N_CORES):
        sl = slice(c * OPC, (c + 1) * OPC)
        in_maps.append({
            "xt": xt,
            "wq": np.ascontiguousarray(wqt[:, sl]),
            "wk": np.ascontiguousarray(wkt[:, sl]),
            "wv": np.ascontiguousarray(wvt[:, sl]),
            "wo": np.ascontiguousarray(wot[sl, :]),
        })

    profile = os.environ.get("KERNEL_PROFILE", "0") == "1"
    try:
        if profile:
            _enable_profiling()
        else:
            _install_hook_shim()
    except Exception:
        profile = False

    nc = _get_nc()
    res = run_bass_kernel_spmd(nc, in_maps, core_ids=list(range(N_CORES)),
                               trace=profile)
    LAST_EXEC_NS = res.exec_time_ns

    total = np.zeros((H, BL), dtype=np.float64)
    for c in range(N_CORES):
        total += res.results[c]["out"]
    return np.ascontiguousarray(total.T).astype(np.float32).reshape(B, L, H)



# revision 7
# speedup vs baseline: 1.0462x; 1.0462x over previous
"""Multi-head causal attention (B=2, L=2048, H=2048, NH=16) on 8 Trainium2
NeuronCores.

Sharding: tensor-parallel over heads — core c computes heads {2c, 2c+1}.
Host sums the 8 partial o-projection outputs.

v2 design (vs the fp32r baseline):
  - all matmul operands in bf16: same PE rate (1 cyc/row, no 256-col
    minimum), half the HBM traffic and SBUF footprint
  - phase-1 (q/k/v proj) and phase-2 (attention) FUSED per 512-token
    chunk: causal attention for chunk g needs only k/v of chunks <= g,
    so the PE stream never has a phase boundary (TRN2 drops to 1.2 GHz
    for ~3us after any PE idle gap)
  - softmax denominator off the PE: DVE accumulates exp tiles into an
    fp32 acc, one gpsimd partition_all_reduce per (chunk, head), DVE
    reciprocal — no ones-matmul, no partition_broadcast
  - causal mask via a static triangular bf16 tile * DVE tensor_mul
    (the local mask is p <= f for every diagonal tile), diagonal
    j-tiles narrowed to their exact live width
  - score matmuls run 2 j-tiles ahead of attnV so ACT exp latency
    never stalls the PE; previous chunk's o-projection is spread as
    PE filler through the attention loop
  - output partials written bf16, 4 o-tiles per DMA, rotated across
    sync/gpsimd queues; host sums in fp32
"""

import os
import sys

if "/opt/trn_rl_repo" not in sys.path:
    sys.path.insert(0, "/opt/trn_rl_repo")

import numpy as np
import ml_dtypes

from concourse import bacc, mybir, tile  # noqa: E402
from concourse.bass_utils import run_bass_kernel_spmd  # noqa: E402
from concourse import bass_isa  # noqa: E402

BF16 = mybir.dt.bfloat16
F32 = mybir.dt.float32

N_CORES = 8
B, L, H, NH = 2, 2048, 2048, 16
DH = H // NH                       # 128
BL = B * L                        # 4096
HPC = NH // N_CORES               # heads per core = 2
OPC = HPC * DH                    # output dims per core = 256
HT = H // 128                     # 16 h-tiles (contraction)
IC = 512                          # chunk width (tokens)
NCH = BL // IC                    # 8 global chunks (4 per batch)
JTB = L // 128                    # 16 j-tiles per batch
SCALE = 1.0 / float(np.sqrt(DH))

LAST_EXEC_NS = None


def _build():
    nc = bacc.Bacc(None, target_bir_lowering=False, debug=False)

    xt = nc.declare_dram_parameter("xt", [H, BL], BF16, isOutput=False)
    wq = nc.declare_dram_parameter("wq", [H, OPC], BF16, isOutput=False)
    wk = nc.declare_dram_parameter("wk", [H, OPC], BF16, isOutput=False)
    wv = nc.declare_dram_parameter("wv", [H, OPC], BF16, isOutput=False)
    wo = nc.declare_dram_parameter("wo", [OPC, H], BF16, isOutput=False)
    out = nc.declare_dram_parameter("out", [H, BL], BF16, isOutput=True)

    with tile.TileContext(nc) as tc:
        with tc.tile_pool(name="persist", bufs=1) as persist, \
             tc.tile_pool(name="psum", bufs=8, space="PSUM") as psum, \
             tc.tile_pool(name="xpool", bufs=2) as xpool, \
             tc.tile_pool(name="qt_pool", bufs=2) as qt_pool, \
             tc.tile_pool(name="exp_pool", bufs=6) as exp_pool, \
             tc.tile_pool(name="acc_pool", bufs=2) as acc_pool, \
             tc.tile_pool(name="rr_pool", bufs=2) as rr_pool, \
             tc.tile_pool(name="mst_pool", bufs=3) as mst_pool, \
             tc.tile_pool(name="oc_pool", bufs=4) as oc_pool:
            kt_sb = persist.tile([128, HPC, BL], BF16, tag="kt")
            v_sb = persist.tile([128, BL // 128, OPC], BF16, tag="v")
            wq_sb = persist.tile([128, HT, OPC], BF16, tag="wq")
            wk_sb = persist.tile([128, HT, OPC], BF16, tag="wk")
            wv_sb = persist.tile([128, HT, OPC], BF16, tag="wv")
            wo_sb = persist.tile([128, HPC, H], BF16, tag="wo")
            tri = persist.tile([128, IC], BF16, tag="tri")

            def dma_split(dst3, src2d):
                # halve a [128, T, F]-tile transfer across both HW queues
                t = dst3.shape[1]
                r = src2d.rearrange("(q t p) f -> q p t f", q=2, p=128)
                nc.sync.dma_start(out=dst3[:, :t // 2, :], in_=r[0])
                nc.scalar.dma_start(out=dst3[:, t // 2:, :], in_=r[1])

            # startup DMAs: wq in per-ht pieces on the gpsimd SWDGE queue
            # so the two HWDGE queues spend the ramp on x/wk/wv
            for ht in range(HT):
                nc.gpsimd.dma_start(
                    out=wq_sb[:, ht, :],
                    in_=wq[ht * 128:(ht + 1) * 128, :])
            xchs = {}
            xchs[0] = xpool.tile([128, HT, IC], BF16, tag="xch", name="xch")
            dma_split(xchs[0], xt[:, 0:IC])
            dma_split(wk_sb, wk[:, :])
            dma_split(wv_sb, wv[:, :])
            nc.gpsimd.dma_start(
                out=wo_sb[:, :, :],
                in_=wo[:, :].rearrange("(t p) f -> p t f", p=128))

            # triangular mask tri[p, f] = 1.0 where p <= f else 0
            tri_f = rr_pool.tile([128, IC], F32, tag="red", name="trif")
            nc.vector.memset(tri_f[:, :], 1.0)
            nc.vector.tensor_copy(tri[:, :], tri_f[:, :])
            nc.gpsimd.affine_select(
                tri[:, :], tri[:, :], pattern=[[1, IC]],
                compare_op=mybir.AluOpType.is_ge, fill=0.0,
                base=0, channel_multiplier=-1)

            # ---- o-projection emitter (for the chunk before `gio`) ----
            oc_state = {"oc": None}

            def emit_oproj_ot(mst, gio, ot):
                op = psum.tile([128, IC], F32, tag="bank", name="op")
                for hh in range(HPC):
                    nc.tensor.matmul(
                        op[:, :],
                        wo_sb[:, hh, ot * 128:(ot + 1) * 128],
                        mst[:, hh, :],
                        start=(hh == 0), stop=(hh == HPC - 1))
                if ot % 4 == 0:
                    oc_state["oc"] = oc_pool.tile([128, 4, IC], BF16,
                                                  tag="oc", name="oc")
                oc = oc_state["oc"]
                if ot % 4 == 1:
                    nc.scalar.copy(oc[:, ot % 4, :], op[:, :])
                else:
                    nc.vector.tensor_copy(oc[:, ot % 4, :], op[:, :])
                if ot % 4 == 3:
                    deng = nc.sync if (ot // 4) % 2 == 0 else nc.gpsimd
                    dst = out[(ot - 3) * 128:(ot + 1) * 128, gio:gio + IC]
                    deng.dma_start(
                        out=dst.rearrange("(t p) f -> p t f", p=128),
                        in_=oc[:, :, :])

            pending = None
            for g in range(NCH):
                b, ic = divmod(g, NCH // B)
                gio = g * IC
                # ---------------- phase 1: q/k/v for chunk g ----------------
                if g + 1 < NCH:
                    xchs[g + 1] = xpool.tile([128, HT, IC], BF16,
                                             tag="xch", name="xch")
                    dma_split(xchs[g + 1], xt[:, (g + 1) * IC:(g + 2) * IC])
                xch = xchs.pop(g)
                qt_g = qt_pool.tile([128, HPC, IC], BF16, tag="qt", name="qt")
                ncopy = 0
                for wsb, dst in ((wq_sb, None), (wk_sb, kt_sb)):
                    for ot in range(HPC):
                        ps = psum.tile([128, IC], F32, tag="bank", name="ps")
                        for ht in range(HT):
                            nc.tensor.matmul(
                                ps[:, :],
                                wsb[:, ht, ot * 128:(ot + 1) * 128],
                                xch[:, ht, :],
                                start=(ht == 0), stop=(ht == HT - 1))
                        dst_ap = (qt_g[:, ot, :] if dst is None
                                  else dst[:, ot, gio:gio + IC])
                        if ncopy % 2 == 0:
                            nc.scalar.copy(dst_ap, ps[:, :])
                        else:
                            nc.vector.tensor_copy(dst_ap, ps[:, :])
                        ncopy += 1
                # v in natural (j x o) layout, two 128-token chains per bank
                for half in range(2):
                    vps = psum.tile([128, 2, OPC], F32, tag="bank", name="ps")
                    for sub in range(2):
                        it = half * 2 + sub
                        for ht in range(HT):
                            nc.tensor.matmul(
                                vps[:, sub, :],
                                xch[:, ht, it * 128:(it + 1) * 128],
                                wv_sb[:, ht, :],
                                start=(ht == 0), stop=(ht == HT - 1))
                    vdst = v_sb[:, g * 4 + half * 2:g * 4 + half * 2 + 2, :]
                    if half == 0:
                        nc.scalar.copy(vdst, vps[:, :, :])
                    else:
                        nc.vector.tensor_copy(vdst, vps[:, :, :])

                # ---------------- phase 2: attention for chunk g ------------
                njt = 4 * ic + 4
                # full-width tile first (initializes acc/mx), then diagonal
                # tiles (their exp->mask chain is longest), then the rest
                order = [0] + list(range(max(1, 4 * ic), 4 * ic + 4)) \
                    + list(range(1, 4 * ic))
                filler = []
                if pending is not None:
                    pmst, pgio = pending
                    filler = [(pmst, pgio, ot) for ot in range(H // 128)]
                fill_idx = 0
                total_iters = HPC * njt
                it_counter = 0
                for h in range(HPC):
                    mx = psum.tile([128, IC], F32, tag="bank", name="mx")
                    acc = acc_pool.tile([128, IC], F32, tag="acc", name="acc")
                    exd = {}

                    def emit_sc_exp(jt):
                        f0 = max(0, 128 * jt - IC * ic)
                        w = IC - f0
                        sc = psum.tile([128, IC], F32, tag="bank", name="sc")
                        nc.tensor.matmul(
                            sc[:, f0:],
                            kt_sb[:, h, b * L + jt * 128:
                                  b * L + (jt + 1) * 128],
                            qt_g[:, h, f0:],
                            start=True, stop=True)
                        ex = exp_pool.tile([128, IC], BF16, tag="ex")
                        nc.scalar.activation(
                            ex[:, f0:], sc[:, f0:],
                            mybir.ActivationFunctionType.Exp,
                            scale=SCALE)
                        if jt >= 4 * ic:
                            nc.vector.tensor_mul(
                                ex[:, f0:], ex[:, f0:], tri[:, :w])
                        exd[jt] = (ex, f0)

                    emit_sc_exp(order[0])
                    if njt > 1:
                        emit_sc_exp(order[1])
                    for idx, jt in enumerate(order):
                        it_counter += 1
                        if idx + 2 < njt:
                            emit_sc_exp(order[idx + 2])
                        ex, f0 = exd.pop(jt)
                        nc.tensor.matmul(
                            mx[:, f0:],
                            v_sb[:, b * JTB + jt, h * 128:(h + 1) * 128],
                            ex[:, f0:],
                            start=(idx == 0), stop=(idx == njt - 1))
                        if idx == 0:
                            nc.vector.tensor_copy(acc[:, :], ex[:, :])
                        else:
                            nc.vector.tensor_add(
                                acc[:, f0:], acc[:, f0:], ex[:, f0:])
                        want = (it_counter * len(filler)) // total_iters
                        while fill_idx < want:
                            emit_oproj_ot(*filler[fill_idx])
                            fill_idx += 1
                    red = rr_pool.tile([128, IC], F32, tag="red", name="red")
                    nc.gpsimd.partition_all_reduce(
                        red[:, :], acc[:, :], channels=128,
                        reduce_op=bass_isa.ReduceOp.add)
                    rec = rr_pool.tile([128, IC], F32, tag="red", name="rec")
                    nc.vector.reciprocal_approx_fast(
                        out=rec[:, :], in_=red[:, :])
                    mst = (mst_pool.tile([128, HPC, IC], BF16, tag="mst",
                                         name="mst")
                           if h == 0 else mst)
                    nc.vector.tensor_mul(mst[:, h, :], mx[:, :], rec[:, :])
                while fill_idx < len(filler):
                    emit_oproj_ot(*filler[fill_idx])
                    fill_idx += 1
                pending = (mst, gio)
            pmst, pgio = pending
            for ot in range(H // 128):
                emit_oproj_ot(pmst, pgio, ot)
    nc.finalize()
    return nc


_NC_CACHE = None


def _get_nc():
    global _NC_CACHE
    if _NC_CACHE is None:
        _NC_CACHE = _build()
    return _NC_CACHE


def _install_hook_shim():
    """Make antenv.axon_hooks importable (absent on this image) so
    run_bass_kernel_spmd's trace path degrades gracefully."""
    import types
    import antenv
    if "antenv.axon_hooks" not in sys.modules:
        shim = types.ModuleType("antenv.axon_hooks")

        def set_axon_ntff_profile_hook(h):
            shim._the_hook = h

        def get_axon_ntff_profile_hook():
            return getattr(shim, "_the_hook", None)

        shim.set_axon_ntff_profile_hook = set_axon_ntff_profile_hook
        shim.get_axon_ntff_profile_hook = get_axon_ntff_profile_hook
        sys.modules["antenv.axon_hooks"] = shim
        antenv.axon_hooks = shim


def _enable_profiling():
    """Wire the axon NTFF profile hook for neuron-profile timing."""
    _install_hook_shim()
    from trn_agent_boot.trn_boot import _ntff_profile_via_ctypes
    hook = _ntff_profile_via_ctypes("/opt/axon/libaxon_pjrt.so")
    sys.modules["antenv.axon_hooks"].set_axon_ntff_profile_hook(hook)
    import concourse.bass_utils as bu
    bu.upload_artifacts = lambda tmpdir: "local://" + tmpdir


def kernel(x, padding_mask, Wq, Wk, Wv, Wo):
    global LAST_EXEC_NS
    bf16 = ml_dtypes.bfloat16
    x = np.asarray(x, dtype=np.float32)

    xt = np.ascontiguousarray(x.reshape(BL, H).T).astype(bf16)   # (H, BL)
    wqt = np.asarray(Wq, dtype=np.float32).T.astype(bf16)        # [h, o]
    wkt = np.asarray(Wk, dtype=np.float32).T.astype(bf16)
    wvt = np.asarray(Wv, dtype=np.float32).T.astype(bf16)
    wot = np.asarray(Wo, dtype=np.float32).T.astype(bf16)        # [h_in, o]

    in_maps = []
    for c in range(N_CORES):
        sl = slice(c * OPC, (c + 1) * OPC)
        in_maps.append({
            "xt": xt,
            "wq": np.ascontiguousarray(wqt[:, sl]),
            "wk": np.ascontiguousarray(wkt[:, sl]),
            "wv": np.ascontiguousarray(wvt[:, sl]),
            "wo": np.ascontiguousarray(wot[sl, :]),
        })

    profile = os.environ.get("KERNEL_PROFILE", "0") == "1"
    try:
        if profile:
            _enable_profiling()
        else:
            _install_hook_shim()
    except Exception:
        profile = False

    nc = _get_nc()
    res = run_bass_kernel_spmd(nc, in_maps, core_ids=list(range(N_CORES)),
                               trace=profile)
    LAST_EXEC_NS = res.exec_time_ns

    total = np.zeros((H, BL), dtype=np.float32)
    for c in range(N_CORES):
        total += np.asarray(res.results[c]["out"], dtype=np.float32)
    return np.ascontiguousarray(total.T).astype(np.float32).reshape(B, L, H)
